# revision 12
# baseline (speedup 1.0000x reference)
"""Trainium2 Bass kernel for nn_AnimationPredictor (2-layer MLP with argmax/one-hot).

Data-parallel over 8 NeuronCores: each core processes 65536 rows.

Math per row (reference):
  h1 = relu(X @ W1.T + b1)            [B, 256]
  logits = h1 @ Wo1.T + bo1           [B, 10]
  y1 = one_hot(argmax(logits), 10)
  h2 = relu(concat([X, y1]) @ W2.T + b2)   [B, 256]
  y2 = sigmoid(h2 @ Wo2.T + bo2)      [B, 6]
  out = concat([y1, y2])              [B, 16]

On-chip layout: batch rows live on the matmul FREE dim ("T layout", features on
partitions) for all big matmuls; X arrives pre-transposed from the host as
fp16 hi/lo pairs (3-term fp16 matmul == f32-grade precision, 1 cyc/row each).
logits are computed in natural layout (rows on partitions) by using h1T column
slices as the stationary operand, so argmax/one-hot run on the DVE along the
free dim. The output is written feature-major [16, rows] and transposed back
on the host.
"""
import sys

sys.path.insert(0, "/opt/trn_rl_repo")

import numpy as np

import concourse.bass as bass
import concourse.tile as tile
from concourse import bacc, mybir
from concourse.bass_utils import run_bass_kernel_spmd

F32 = mybir.dt.float32
FP16 = mybir.dt.float16

N_CORES = 8
BATCH = 524288
IN = 128
H = 256
O1 = 10
O2 = 6
SHARD = BATCH // N_CORES          # 65536 rows per core
MACRO = 512                       # rows per macro-tile (one PSUM bank at f32)
SUB = 128                         # rows per subtile (stationary M limit)
NSUB = MACRO // SUB               # 4
GSTRIDE = 32                      # per-subtile group stride in the packed logits tile
NEG_BIG = -3.0e38


def build(n_macros=SHARD // MACRO):
    nc = bacc.Bacc("TRN2", target_bir_lowering=False, debug=False)
    rows = n_macros * MACRO

    # --- DRAM parameters (per-core shapes) ---
    xt_hi = nc.dram_tensor("xt_hi", [IN, rows], FP16, kind="ExternalInput").ap()
    xt_lo = nc.dram_tensor("xt_lo", [IN, rows], FP16, kind="ExternalInput").ap()
    w1t_hi = nc.dram_tensor("w1t_hi", [IN, H], FP16, kind="ExternalInput").ap()
    w1t_lo = nc.dram_tensor("w1t_lo", [IN, H], FP16, kind="ExternalInput").ap()
    b1_d = nc.dram_tensor("b1", [128, 2], F32, kind="ExternalInput").ap()
    wo1t_d = nc.dram_tensor("wo1t", [128, 2 * O1], F32, kind="ExternalInput").ap()
    bo1_d = nc.dram_tensor("bo1", [128, O1], F32, kind="ExternalInput").ap()
    w2xt_d = nc.dram_tensor("w2xt", [IN, H], FP16, kind="ExternalInput").ap()
    w2yt_d = nc.dram_tensor("w2yt", [O1, H], FP16, kind="ExternalInput").ap()
    b2_d = nc.dram_tensor("b2", [128, 2], F32, kind="ExternalInput").ap()
    wo2t_d = nc.dram_tensor("wo2t", [128, 2 * O2], FP16, kind="ExternalInput").ap()
    bo2_d = nc.dram_tensor("bo2", [O2, 1], F32, kind="ExternalInput").ap()
    eye16_d = nc.dram_tensor("eye16", [128, 128], FP16, kind="ExternalInput").ap()
    outT = nc.dram_tensor("outT", [O1 + O2, rows], F32, kind="ExternalOutput").ap()

    with tile.TileContext(nc) as tc:
        with tc.tile_pool(name="const", bufs=1) as cpool, \
             tc.tile_pool(name="xin", bufs=3) as xin, \
             tc.tile_pool(name="h1sb", bufs=4) as h1sb, \
             tc.tile_pool(name="small", bufs=2) as small, \
             tc.tile_pool(name="h2sb", bufs=4) as h2sb, \
             tc.tile_pool(name="h1ps", bufs=2, space="PSUM") as h1ps, \
             tc.tile_pool(name="lgps", bufs=2, space="PSUM") as lgps, \
             tc.tile_pool(name="y1ps", bufs=1, space="PSUM") as y1ps, \
             tc.tile_pool(name="h2ps", bufs=2, space="PSUM") as h2ps, \
             tc.tile_pool(name="y2ps", bufs=1, space="PSUM") as y2ps:

            # --- constants into SBUF ---
            w1t_hi_sb = cpool.tile_from(w1t_hi)
            w1t_lo_sb = cpool.tile_from(w1t_lo)
            b1_sb = cpool.tile_from(b1_d)
            wo1t_sb = cpool.tile_from(wo1t_d)
            bo1_sb = cpool.tile_from(bo1_d)
            w2xt_sb = cpool.tile_from(w2xt_d)
            w2yt_sb = cpool.tile_from(w2yt_d)
            b2_sb = cpool.tile_from(b2_d)
            wo2t_sb = cpool.tile_from(wo2t_d)
            bo2_sb = cpool.tile_from(bo2_d)
            eye16_sb = cpool.tile_from(eye16_d)

            for m in range(n_macros):
                c0 = m * MACRO
                # --- load X.T tiles (fp16 hi/lo) ---
                xh = xin.tile([IN, MACRO], FP16, tag="xh")
                nc.sync.dma_start(xh[:], xt_hi[:, c0:c0 + MACRO])
                xl = xin.tile([IN, MACRO], FP16, tag="xl")
                nc.sync.dma_start(xl[:], xt_lo[:, c0:c0 + MACRO])

                # --- stage 1: h1T = relu(W1 @ X.T + b1), fp16 3-term ---
                h1t = []
                for c in range(2):
                    ps = h1ps.tile([128, MACRO], F32, tag="h1ps")
                    wh = w1t_hi_sb[:, 128 * c:128 * (c + 1)]
                    wl = w1t_lo_sb[:, 128 * c:128 * (c + 1)]
                    nc.tensor.matmul(ps[:], wh, xh[:], start=True, stop=False)
                    nc.tensor.matmul(ps[:], wh, xl[:], start=False, stop=False)
                    nc.tensor.matmul(ps[:], wl, xh[:], start=False, stop=True)
                    sb = h1sb.tile([128, MACRO], F32, tag="h1")
                    # relu: (psum + b1) max 0
                    nc.vector.tensor_scalar(
                        sb[:], ps[:], b1_sb[:, c:c + 1], 0.0,
                        mybir.AluOpType.add, mybir.AluOpType.max)
                    h1t.append(sb)

                # --- logits (natural layout), f32 ---
                lg = lgps.tile([128, 128], F32, tag="lg")
                for s in range(NSUB):
                    for c in range(2):
                        nc.tensor.matmul(
                            lg[:, GSTRIDE * s:GSTRIDE * s + O1],
                            h1t[c][:, SUB * s:SUB * (s + 1)],
                            wo1t_sb[:, O1 * c:O1 * (c + 1)],
                            start=(c == 0), stop=(c == 1))

                # packed logits + bo1 (groups of 32, 10 valid cols each)
                packed = small.tile([128, 128], F32, tag="packed")
                pk3 = packed[:].rearrange("p (g c) -> p g c", c=GSTRIDE)[:, :, 0:O1]
                lg3 = lg[:].rearrange("p (g c) -> p g c", c=GSTRIDE)[:, :, 0:O1]
                bo1_b = bo1_sb[:].unsqueeze(1).broadcast_to([128, NSUB, O1])
                nc.vector.tensor_tensor(pk3, lg3, bo1_b, mybir.AluOpType.add)

                # --- argmax -> one-hot (exact f32 compare) ---
                mx = small.tile([128, NSUB], F32, tag="mx")
                nc.vector.tensor_reduce(
                    out=mx[:], in_=pk3, op=mybir.AluOpType.max,
                    axis=mybir.AxisListType.X)
                oh = small.tile([128, 128], FP16, tag="oh")
                nc.vector.memset(oh[:], 0.0)
                oh3 = oh[:].rearrange("p (g c) -> p g c", c=GSTRIDE)[:, :, 0:O1]
                mx_b = mx[:].unsqueeze(2).broadcast_to([128, NSUB, O1])
                nc.vector.tensor_tensor(oh3, pk3, mx_b, mybir.AluOpType.is_equal)

                # --- transpose one-hot -> unified y1T [10, MACRO] ---
                y1ps_t = y1ps.tile([O1, MACRO], FP16, tag="y1ps")
                for s in range(NSUB):
                    nc.tensor.transpose(
                        y1ps_t[:, SUB * s:SUB * (s + 1)],
                        oh[:, GSTRIDE * s:GSTRIDE * s + O1], eye16_sb[:])
                y1t = small.tile([O1, MACRO], FP16, tag="y1t")
                nc.vector.tensor_copy(y1t[:], y1ps_t[:])
                y1f = small.tile([O1, MACRO], F32, tag="y1f")
                nc.vector.tensor_copy(y1f[:], y1ps_t[:])

                # --- stage 2: h2T = relu(W2x @ X.T + W2y @ y1T + b2), fp16 ---
                h2t = []
                for c in range(2):
                    ps = h2ps.tile([128, MACRO], F32, tag="h2ps")
                    nc.tensor.matmul(
                        ps[:], w2xt_sb[:, 128 * c:128 * (c + 1)], xh[:],
                        start=True, stop=False)
                    nc.tensor.matmul(
                        ps[:], w2yt_sb[:, 128 * c:128 * (c + 1)], y1t[:],
                        start=False, stop=True)
                    sb = h2sb.tile([128, MACRO], FP16, tag="h2")
                    nc.vector.tensor_scalar(
                        sb[:], ps[:], b2_sb[:, c:c + 1], 0.0,
                        mybir.AluOpType.add, mybir.AluOpType.max)
                    h2t.append(sb)

                # --- y2T = sigmoid(Wo2 @ h2T + bo2) ---
                y2p = y2ps.tile([O2, MACRO], F32, tag="y2ps")
                for c in range(2):
                    nc.tensor.matmul(
                        y2p[:], wo2t_sb[:, O2 * c:O2 * (c + 1)], h2t[c][:],
                        start=(c == 0), stop=(c == 1))
                y2t = small.tile([O2, MACRO], F32, tag="y2t")
                nc.scalar.activation(
                    y2t[:], y2p[:], mybir.ActivationFunctionType.Sigmoid,
                    bias=bo2_sb[:, 0:1], scale=1.0)

                # --- outputs (feature-major) ---
                nc.sync.dma_start(outT[O1:O1 + O2, c0:c0 + MACRO], y2t[:])
                nc.sync.dma_start(outT[0:O1, c0:c0 + MACRO], y1f[:])
    nc.compile()
    return nc


def _prep_inputs(X, W1, b1, Wo1, bo1, W2, b2, Wo2, bo2, rows_per_core, n_cores):
    """Host-side prep: shard + transpose X, split fp16 hi/lo, pack weights."""
    X = np.asarray(X, dtype=np.float32)
    W1 = np.asarray(W1, dtype=np.float32)
    b1 = np.asarray(b1, dtype=np.float32)
    Wo1 = np.asarray(Wo1, dtype=np.float32)
    bo1 = np.asarray(bo1, dtype=np.float32)
    W2 = np.asarray(W2, dtype=np.float32)
    b2 = np.asarray(b2, dtype=np.float32)
    Wo2 = np.asarray(Wo2, dtype=np.float32)
    bo2 = np.asarray(bo2, dtype=np.float32)

    w1t = np.ascontiguousarray(W1.T)                     # [128, 256]
    w1t_hi = w1t.astype(np.float16)
    w1t_lo = (w1t - w1t_hi.astype(np.float32)).astype(np.float16)
    w2t = W2.T                                           # [138, 256]
    w2xt = np.ascontiguousarray(w2t[:IN]).astype(np.float16)
    w2yt = np.ascontiguousarray(w2t[IN:]).astype(np.float16)
    wo1t = np.ascontiguousarray(Wo1.T)                   # [256, 10]
    wo1t_p = np.concatenate([wo1t[:128], wo1t[128:]], axis=1)  # [128, 20]
    wo2t = np.ascontiguousarray(Wo2.T).astype(np.float16)      # [256, 6]
    wo2t_p = np.concatenate([wo2t[:128], wo2t[128:]], axis=1)  # [128, 12]

    common = {
        "w1t_hi": w1t_hi, "w1t_lo": w1t_lo,
        "b1": np.ascontiguousarray(b1.reshape(2, 128).T),
        "wo1t": wo1t_p,
        "bo1": np.ascontiguousarray(np.broadcast_to(bo1, (128, O1))),
        "w2xt": w2xt, "w2yt": w2yt,
        "b2": np.ascontiguousarray(b2.reshape(2, 128).T),
        "wo2t": wo2t_p,
        "bo2": np.ascontiguousarray(bo2.reshape(O2, 1)),
        "eye16": np.eye(128, dtype=np.float16),
    }

    in_maps = []
    for c in range(n_cores):
        Xs = X[c * rows_per_core:(c + 1) * rows_per_core]
        xt = np.ascontiguousarray(Xs.T)                  # [128, rows]
        hi = xt.astype(np.float16)
        lo = (xt - hi.astype(np.float32)).astype(np.float16)
        in_maps.append({**common, "xt_hi": hi, "xt_lo": lo})
    return in_maps


_NC_CACHE = {}


def _get_nc(n_macros):
    if n_macros not in _NC_CACHE:
        _NC_CACHE[n_macros] = build(n_macros)
    return _NC_CACHE[n_macros]


def run(X, W1, b1, Wo1, bo1, W2, b2, Wo2, bo2, trace=False):
    """Full-size run across 8 cores. Returns (out [B,16] f32, exec_time_ns|None)."""
    n_macros = SHARD // MACRO
    nc = _get_nc(n_macros)
    in_maps = _prep_inputs(X, W1, b1, Wo1, bo1, W2, b2, Wo2, bo2, SHARD, N_CORES)
    res = run_bass_kernel_spmd(nc, in_maps, core_ids=list(range(N_CORES)), trace=trace)
    out = np.empty((BATCH, O1 + O2), dtype=np.float32)
    for c in range(N_CORES):
        out[c * SHARD:(c + 1) * SHARD] = res.results[c]["outT"].T
    return out, res.exec_time_ns


def kernel(X, W1, b1, Wo1, bo1, W2, b2, Wo2, bo2):
    out, _ = run(X, W1, b1, Wo1, bo1, W2, b2, Wo2, bo2)
    return out
